# revision 12
# baseline (speedup 1.0000x reference)
"""Trainium2 kernel for nn_ContrasiveLoss (segment-reduce contrastive loss).

Strategy (data-parallel, one image per NeuronCore, 8 cores):
  The per-image loss only needs per-segment statistics
      counts[k], sums[k, c], sumsq[k] = sum_{p in k} ||f_p||^2
  because the variance term telescopes:
      sum_{p in k} ||f_p - mean_k||^2 = sumsq_k - counts_k * ||mean_k||^2.

  The host sorts each image's pixels by label and pads every segment to a
  fixed NWSEG * 512 pixels (zero features), so every 512-pixel window
  belongs to exactly one segment and the window -> segment schedule is
  data-independent (same program on all 8 cores).  Pixels are dealt into a
  [128 partitions x 4 groups] block per window; features are stored fp8
  (e3m4).  One matmul per window computes the Gram matrix of the window's
  feature block against itself plus a ones column:
      psum[k] += [F^T F | F^T 1]          (F = [128 pix, 128 = 4g x 32c])
  whose diagonal accumulates per-(group,channel) sum-of-squares and whose
  ones column accumulates per-(group,channel) sums -- the entire segment
  reduction runs on the TensorEngine; no bulk vector-engine work, which is
  what makes fp8 viable (DVE has no fp8 fast path).

  Counts come from a host-precomputed graded mask column (value per
  partition = #valid group-slots, 0..4), reduced per segment on DVE.
  Per segment, a fused tensor_tensor_reduce extracts the Gram diagonal sum
  as one column of a [128, 48] tile; a single PE transpose then yields
  segment-major rows and a short epilogue computes
      (var + hinge-dist + gamma * reg) / K
  on-chip.  The host sums the 8 per-image scalars and divides by (B+1).
"""

import numpy as np
import ml_dtypes

import concourse.bass as bass
import concourse.mybir as mybir
import concourse.tile as tile
from concourse.bass_utils import run_bass_kernel_spmd
from concourse.vector_clock import ScopedClock

# ---------------------------------------------------------------- problem dims
B, C, H, W = 8, 32, 512, 512
K = 16
N = H * W                # pixels per image
G = 4                    # pixel groups packed alongside channels (4*32 = 128)
PIXW = 128 * G           # pixels per window
WSTRIDE = 132            # per-window cols: [f 128 | ones 1 | pad 3] (4B aligned)
FCOLS = 128

DD = 2.5
GAMMA = 0.005

FDT = mybir.dt.float8e3
FDT_NP = ml_dtypes.float8_e3m4
FP32 = mybir.dt.float32

TRACE = False            # test harness flips this for NTFF profiling


# ------------------------------------------------- container-specific patches
def _patch_tile_drain() -> None:
    """This container's walrus build accepts only ONE sync-wait command per
    instruction, but TileContext's tail drain attaches one wait per active
    semaphore lane.  Split the tail drain into a chain of single-wait drains.
    """
    if getattr(tile.TileContext, "_drain_split_patched", False):
        return

    def _drain_and_barrier(self, tick_clock, wait_clock):
        drain_inst = self.nc.sync.drain()
        wait_clock.add_sem_waits(
            drain_inst.ins, ScopedClock({None: tick_clock.global_clock})
        )
        si = drain_inst.ins.sync_info
        if si is not None and len(si.on_wait) > 1:
            waits = list(si.on_wait)
            drain_inst.ins.sync_info = mybir.SyncInfo(
                on_wait=[waits[0]], on_update=list(si.on_update)
            )
            for w in waits[1:]:
                d2 = self.nc.sync.drain()
                d2.ins.sync_info = mybir.SyncInfo(on_wait=[w], on_update=[])

        self.nc.all_engine_barrier()
        assert self.sems is not None
        popped = self.nc._tile_sem_poison_stack.pop()
        assert popped is self._sem_poison
        self.nc.clear_and_free_semaphores(list(self.sems.allocated().values()))
        self.nc.all_engine_barrier()

    tile.TileContext._drain_and_barrier = _drain_and_barrier
    tile.TileContext._drain_split_patched = True


def _split_multi_waits(nc) -> None:
    """Walrus accepts one sync-wait per instruction: hoist extra waits onto
    single-wait Drain instructions on the same engine, inserted just before."""
    for fn in nc.m.functions:
        for blk in fn.blocks:
            changed = False
            out = []
            for ins in blk.instructions:
                si = ins.sync_info
                if si is not None and len(si.on_wait) > 1:
                    changed = True
                    waits = list(si.on_wait)
                    for j, w in enumerate(waits[:-1]):
                        d = mybir.InstDrain(name=f"{ins.name}-ws{j}")
                        d.engine = ins.engine
                        d.sync_info = mybir.SyncInfo(on_wait=[w], on_update=[])
                        out.append(d)
                    ins.sync_info = mybir.SyncInfo(
                        on_wait=[waits[-1]], on_update=list(si.on_update)
                    )
                out.append(ins)
            if changed:
                blk.instructions = out


# ------------------------------------------------------------- device program
def _build_kernel(nwseg: int):
    _patch_tile_drain()
    nc = bass.Bass("TRN2")
    nw = K * nwseg

    fmov = nc.dram_tensor("fmov", [128, nw * WSTRIDE], FDT, kind="ExternalInput")
    masks = nc.dram_tensor("masks", [128, nw], FP32, kind="ExternalInput")
    out = nc.dram_tensor("out", [1, 1], FP32, kind="ExternalOutput")


    with tile.TileContext(nc) as tc:
        with (
            tc.tile_pool(name="consts", bufs=1) as consts,
            tc.tile_pool(name="feat", bufs=2) as featp,
            tc.tile_pool(name="scr", bufs=2) as scrp,
            tc.tile_pool(name="gram", bufs=3, space="PSUM") as gramp,
            tc.tile_pool(name="eps", bufs=1, space="PSUM") as epsp,
            tc.tile_pool(name="epi", bufs=1) as epi,
        ):
            # ---- constants built on-device (no slow inline loads)
            sb_ones = consts.tile([128, 128], FP32)
            nc.vector.memset(sb_ones, 1.0)
            # identity / diagonal mask: keep where p - c == 0
            sb_id128 = consts.tile([128, 128], FP32)
            nc.gpsimd.affine_select(
                out=sb_id128, in_=sb_ones, pattern=[[-1, 128]],
                compare_op=mybir.AluOpType.is_equal, fill=0.0,
                base=0, channel_multiplier=1,
            )
            # strict upper-triangular 16x16: keep where c - p - 1 >= 0
            sb_triu = consts.tile([16, 16], FP32)
            nc.gpsimd.affine_select(
                out=sb_triu, in_=sb_ones[0:16, 0:16], pattern=[[1, 16]],
                compare_op=mybir.AluOpType.is_ge, fill=0.0,
                base=-1, channel_multiplier=-1,
            )
            sb_diagmask = sb_id128
            sb_id16 = sb_id128[0:16, 0:16]
            sb_ones_row = sb_ones[0:1, 0:16]
            sb_ones_col = sb_ones[0:16, 0:1]
            sb_masks = consts.tile([128, nw], FP32)
            nc.sync.dma_start(out=sb_masks, in_=masks[:, :])

            # stats columns (32-aligned blocks): [diag 0:16 | sums 32:48 | cnt 64:80]
            t3 = epi.tile([128, 80], FP32)

            seglen = nwseg * WSTRIDE
            half = (nwseg // 2) * WSTRIDE
            for k in range(K):
                fseg = featp.tile(
                    [128, seglen], FDT, name=f"fseg{k}", tag="fseg"
                )
                a = k * seglen
                nc.sync.dma_start(out=fseg[:, 0:half], in_=fmov[:, a:a + half])
                nc.sync.dma_start(
                    out=fseg[:, half:seglen], in_=fmov[:, a + half:a + seglen]
                )
                psum_k = gramp.tile([128, 129], FP32, name=f"psum{k}",
                                    tag="gram")
                for j in range(nwseg):
                    base = j * WSTRIDE
                    nc.tensor.matmul(
                        psum_k[:, :],
                        fseg[:, base:base + FCOLS],
                        fseg[:, base:base + FCOLS + 1],
                        start=(j == 0), stop=(j == nwseg - 1),
                    )
                # diag(Gram) summed per partition -> t3 col k
                scratch = scrp.tile([128, 128], FP32, name=f"scr{k}", tag="scr")
                nc.vector.tensor_mul(scratch, psum_k[:, 0:128], sb_diagmask)
                nc.vector.tensor_reduce(
                    out=t3[:, k:k + 1], in_=scratch,
                    axis=mybir.AxisListType.X, op=mybir.AluOpType.add,
                )
                # sums column -> t3 col 16+k  (ScalarE: closer to PSUM)
                nc.scalar.copy(out=t3[:, 32 + k:33 + k], in_=psum_k[:, 128:129])
                # window-validity mask -> per-partition count -> t3 col 32+k
                nc.vector.tensor_reduce(
                    out=t3[:, 64 + k:65 + k],
                    in_=sb_masks[:, k * nwseg:(k + 1) * nwseg],
                    axis=mybir.AxisListType.X,
                    op=mybir.AluOpType.add,
                )

            # ================= epilogue: stats -> scalar loss =================
            psum_t = epsp.tile([80, 128], FP32)
            nc.tensor.transpose(psum_t[:, :], t3, sb_id128)
            sqk = epi.tile([16, 1], FP32)
            nc.vector.tensor_reduce(
                out=sqk, in_=psum_t[0:16, :], axis=mybir.AxisListType.X,
                op=mybir.AluOpType.add,
            )
            counts = epi.tile([16, 1], FP32)
            nc.vector.tensor_reduce(
                out=counts, in_=psum_t[64:80, :], axis=mybir.AxisListType.X,
                op=mybir.AluOpType.add,
            )
            s128 = epi.tile([16, 128], FP32)
            nc.scalar.copy(out=s128, in_=psum_t[32:48, :])
            t64a = epi.tile([16, 64], FP32)
            nc.vector.tensor_add(t64a, s128[:, 0:64], s128[:, 64:128])
            sums = epi.tile([16, 32], FP32)
            nc.vector.tensor_add(sums, t64a[:, 0:32], t64a[:, 32:64])

            recip = epi.tile([16, 1], FP32)
            nc.vector.reciprocal(out=recip, in_=counts)

            means = epi.tile([16, 32], FP32)
            nc.vector.tensor_scalar_mul(out=means, in0=sums, scalar1=recip)
            msq = epi.tile([16, 32], FP32)
            nc.vector.tensor_mul(msq, means, means)
            m2 = epi.tile([16, 1], FP32)
            nc.vector.tensor_reduce(
                out=m2, in_=msq, axis=mybir.AxisListType.X,
                op=mybir.AluOpType.add,
            )
            vark = epi.tile([16, 1], FP32)
            nc.vector.tensor_scalar(
                out=vark, in0=sqk, scalar1=recip, scalar2=m2,
                op0=mybir.AluOpType.mult, op1=mybir.AluOpType.subtract,
            )

            # pairwise distances: diff2 = m2_i + m2_j - 2 * means @ means.T
            psumT = epsp.tile([32, 16], FP32)
            nc.tensor.transpose(psumT[:, :], means, sb_id16)
            meansT = epi.tile([32, 16], FP32)
            nc.vector.tensor_copy(meansT, psumT)
            meansTn2 = epi.tile([32, 16], FP32)
            nc.vector.tensor_scalar_mul(out=meansTn2, in0=meansT, scalar1=-2.0)

            psumR = epsp.tile([1, 16], FP32)
            nc.tensor.transpose(psumR[:, :], m2, sb_id16)
            m2row = epi.tile([1, 16], FP32)
            nc.vector.tensor_copy(m2row, psumR)

            psumD = epsp.tile([16, 16], FP32)
            nc.tensor.matmul(psumD[:, :], sb_ones_row, m2row,
                             start=True, stop=False)
            nc.tensor.matmul(psumD[:, :], m2row, sb_ones_row,
                             start=False, stop=False)
            nc.tensor.matmul(psumD[:, :], meansTn2, meansT,
                             start=False, stop=True)

            diff2 = epi.tile([16, 16], FP32)
            nc.vector.tensor_scalar_max(out=diff2, in0=psumD, scalar1=0.0)
            dist = epi.tile([16, 16], FP32)
            nc.scalar.activation(out=dist, in_=diff2,
                                 func=mybir.ActivationFunctionType.Sqrt)
            regk = epi.tile([16, 1], FP32)
            nc.scalar.activation(out=regk, in_=m2,
                                 func=mybir.ActivationFunctionType.Sqrt)

            hinge = epi.tile([16, 16], FP32)
            nc.vector.tensor_scalar(
                out=hinge, in0=dist, scalar1=-1.0, scalar2=2.0 * DD,
                op0=mybir.AluOpType.mult, op1=mybir.AluOpType.add,
            )
            nc.vector.tensor_scalar_max(out=hinge, in0=hinge, scalar1=0.0)
            nc.vector.tensor_mul(hinge, hinge, hinge)
            nc.vector.tensor_mul(hinge, hinge, sb_triu)

            hrow = epi.tile([16, 1], FP32)
            nc.vector.tensor_reduce(
                out=hrow, in_=hinge, axis=mybir.AxisListType.X,
                op=mybir.AluOpType.add,
            )
            # per-segment combine: vark + GAMMA*regk + hrow/(K-1), then /K
            comb = epi.tile([16, 1], FP32)
            nc.vector.tensor_scalar(
                out=comb, in0=regk, scalar1=GAMMA, scalar2=vark,
                op0=mybir.AluOpType.mult, op1=mybir.AluOpType.add,
            )
            nc.vector.tensor_scalar(
                out=comb, in0=hrow, scalar1=1.0 / (K - 1), scalar2=comb,
                op0=mybir.AluOpType.mult, op1=mybir.AluOpType.add,
            )
            psumS = epsp.tile([1, 1], FP32)
            nc.tensor.matmul(psumS[:, :], sb_ones_col, comb,
                             start=True, stop=True)
            loss = epi.tile([1, 1], FP32)
            nc.vector.tensor_scalar(
                out=loss, in0=psumS, scalar1=1.0 / K, scalar2=None,
                op0=mybir.AluOpType.mult,
            )
            nc.sync.dma_start(out=out[:, :], in_=loss)

    _split_multi_waits(nc)
    return nc


_NC_CACHE = {}


def _get_kernel(nwseg: int):
    if nwseg not in _NC_CACHE:
        _NC_CACHE[nwseg] = _build_kernel(nwseg)
    return _NC_CACHE[nwseg]


# --------------------------------------------------------------- entry point
def _marshal_image(feat: np.ndarray, lab: np.ndarray, nwseg: int):
    """feat [C, H, W] f32, lab [H, W] int -> (fmov [128, nw*WSTRIDE] fp8,
    masks [128, nw] f32) with pixels sorted by label, each segment padded to
    nwseg windows of 512 pixels, dealt g-minor into [128 part x 4 grp]."""
    nw = K * nwseg
    L = nwseg * PIXW
    l = lab.reshape(N).astype(np.int64)
    counts = np.bincount(l, minlength=K)
    assert counts.max() <= L

    order = np.argsort(l, kind="stable")
    cum = np.zeros(K, dtype=np.int64)
    cum[1:] = np.cumsum(counts)[:-1]
    # rank of each sorted pixel within its segment
    within = np.arange(N, dtype=np.int64) - np.repeat(cum, counts)
    pos = np.repeat(np.arange(K, dtype=np.int64), counts) * L + within

    padded = np.zeros((K * L, C), dtype=np.float32)
    padded[pos] = feat.reshape(C, N).T[order]
    q = padded.astype(FDT_NP)

    # slot (w, p, g) = global index 512w + 4p + g
    qw = q.reshape(nw, 128, G, C)
    fblock = np.ascontiguousarray(qw.transpose(1, 0, 2, 3)).reshape(
        128, nw, G * C
    )
    fmov = np.zeros((128, nw, WSTRIDE), dtype=FDT_NP)
    fmov[:, :, 0:FCOLS] = fblock
    fmov[:, :, FCOLS] = np.float32(1.0)

    # graded validity mask: row p covers slots 4p..4p+3 of the window
    wseg = np.arange(nw) // nwseg
    wloc = np.arange(nw) % nwseg
    vw = np.clip(counts[wseg] - wloc * PIXW, 0, PIXW)          # [nw]
    m = np.clip(vw[None, :] - 4 * np.arange(128)[:, None], 0, G)
    return fmov.reshape(128, nw * WSTRIDE), m.astype(np.float32)


def kernel(features_batch, labels_batch, num_instances):
    assert int(num_instances) == K
    features_batch = np.asarray(features_batch, dtype=np.float32)
    labels_batch = np.asarray(labels_batch)
    assert features_batch.shape == (B, C, H, W)

    allc = np.stack(
        [np.bincount(labels_batch[b].reshape(N), minlength=K)
         for b in range(B)]
    )
    nwseg = max(33, int(-(-allc.max() // PIXW)))

    nc = _get_kernel(nwseg)
    in_maps = []
    for i in range(B):
        fmov, m = _marshal_image(features_batch[i], labels_batch[i], nwseg)
        in_maps.append({"fmov": fmov, "masks": m})

    res = run_bass_kernel_spmd(
        nc, in_maps, core_ids=list(range(B)), trace=TRACE
    )
    kernel.last_result = res
    losses = [res.results[i]["out"][0, 0] for i in range(B)]
    total = np.float64(0.0)
    for v in losses:
        total += np.float64(v)
    return np.array(total / (B + 1), dtype=np.float32)
